# revision 6
# baseline (speedup 1.0000x reference)
"""Trainium2 Bass kernel for masked-softmax attention scoring.

Reference computation (B=128, T=512, K=1024, Q=1024):
    mids  = einsum("kq,bq->bk", W, query)
    s     = tanh(einsum("btk,bk->bt", key, mids) + bias)
    attn  = softmax-like: exp(s - max) * mask / sum(exp(s - max) * mask)

The max-subtraction cancels exactly in the ratio (tanh is bounded), so the
device computes  attn = exp(tanh(.)) * mask / sum_t(exp(tanh(.)) * mask).

Sharding: data-parallel over B across 8 NeuronCores (16 batches/core).
Per-core layout: partition p = (b, j) with b in [0,16), j in [0,8);
free column c in [0,64); timestep t = j*64 + c.

v2 design (DMA-bound pipeline, ~420 GB/s/core fabric):
 - W^T prologue split across both HWDGE rings, mids matmul in chunk
   arrival order (stationary = 8x-replicated query columns, f32r).
 - key streams as 64 per-column 512 KB DMAs alternating rings into a
   20-deep column pool; consumption is the NATIVE fused
   scalar_tensor_tensor (out = key*mids, accum_out = per-col score):
   DVE takes 3 of every 4 columns (~1.2 us/col), GpSimd every 4th
   (~2.5 us/col) so combined consumption always outruns arrival and the
   DMA rings never stall on pool recycling.
 - tanh/exp run incrementally per 16-column group on the ACT engine;
   the final mask-multiply + row-sum is one DVE op, group-sum via
   block-diagonal 0/1 matmul, reciprocal, scale, store.
"""

import sys

if "/opt/trn_rl_repo" not in sys.path:
    sys.path.insert(0, "/opt/trn_rl_repo")

from contextlib import ExitStack

import numpy as np

# ---- problem constants (hardcoded per spec) ----
B, T, K, Q = 128, 512, 1024, 1024
NCORES = 8
BS = B // NCORES          # 16 batches per core
P = 128                   # SBUF partitions
J = P // BS               # 8 t-blocks per batch on partitions
CF = T // J               # 64 timesteps per (partition, free col)
QC = Q // P               # 8 contraction chunks for the mids matmul
KEY_BUFS = 20             # key column pool depth (512 KB per slot)
GP_MOD = 4                # gpsimd consumes cols with c % GP_MOD == GP_MOD-1
GP_MAX = 60               # ... but only below this col (DVE takes the tail)

_STATE: dict = {}


def _build_nc():
    import concourse.tile as tile
    from concourse import bacc, mybir

    f32 = mybir.dt.float32
    f32r = mybir.dt.float32r
    mul = mybir.AluOpType.mult
    byp = mybir.AluOpType.bypass
    nc = bacc.Bacc()

    qt_e = nc.declare_dram_parameter("qt", [P, QC, BS], f32r, isOutput=False)
    wt_e = nc.declare_dram_parameter("wt", [P, QC, K], f32r, isOutput=False)
    grp_e = nc.declare_dram_parameter("grp", [P, P], f32, isOutput=False)
    key_e = nc.declare_dram_parameter("key", [BS, T, K], f32, isOutput=False)
    maskr_e = nc.declare_dram_parameter("maskr", [P, CF], f32, isOutput=False)
    bias_e = nc.declare_dram_parameter("biasb", [P, 1], f32, isOutput=False)
    out_e = nc.declare_dram_parameter("out", [P, CF], f32, isOutput=True)

    with tile.TileContext(nc) as tc, ExitStack() as ctx:
        const = ctx.enter_context(tc.tile_pool(name="const", bufs=1))
        kpool = ctx.enter_context(tc.tile_pool(name="key", bufs=KEY_BUFS))
        vpool = ctx.enter_context(tc.tile_pool(name="vprod", bufs=2))
        gpool = ctx.enter_context(tc.tile_pool(name="gprod", bufs=2))
        psum = ctx.enter_context(tc.tile_pool(name="psum", bufs=1, space="PSUM"))

        # ---- prologue loads split across BOTH HWDGE rings so W^T lands
        # at aggregate fabric bandwidth; key columns queue right behind.
        qt_sb = const.tile([P, QC, BS], f32r)
        nc.sync.dma_start(out=qt_sb[:], in_=qt_e[:])
        maskr_sb = const.tile([P, CF], f32)
        nc.scalar.dma_start(out=maskr_sb[:], in_=maskr_e[:])
        bias_sb = const.tile([P, 1], f32)
        nc.scalar.dma_start(out=bias_sb[:], in_=bias_e[:])
        grp_sb = const.tile([P, P], f32)
        nc.scalar.dma_start(out=grp_sb[:], in_=grp_e[:])
        wt_sb = const.tile([P, QC, K], f32r)
        for qc in range(QC // 2):
            nc.sync.dma_start(out=wt_sb[:, qc, :], in_=wt_e[:, qc, :])
        for qc in range(QC // 2, QC):
            nc.scalar.dma_start(out=wt_sb[:, qc, :], in_=wt_e[:, qc, :])

        # ---- mids in broadcast layout: [P, K], row p = mids[b(p), :] ----
        # Replicate each query column 8x on-chip (stride-0 DVE read) so the
        # stationary operand has the (b, j) partition order in one free dim.
        qtrep_sb = const.tile([P, QC, BS, J], f32r)
        nc.gpsimd.tensor_copy(
            qtrep_sb[:], qt_sb[:].unsqueeze(-1).broadcast_to((P, QC, BS, J))
        )
        # matmuls in wt-chunk ARRIVAL order (rings deliver 0..3 and 4..7
        # concurrently); PSUM accumulation order is numerically immaterial.
        mids_ps = psum.tile([P, K], f32)
        qc_order = [0, 4, 1, 5, 2, 6, 3, 7]
        for qi, qc in enumerate(qc_order):
            lhsT = qtrep_sb[:, qc, :, :]
            for h in range(2):
                nc.tensor.matmul(
                    mids_ps[:, h * 512 : (h + 1) * 512],
                    lhsT=lhsT,
                    rhs=wt_sb[:, qc, h * 512 : (h + 1) * 512],
                    start=(qi == 0),
                    stop=(qi == QC - 1),
                )
        mids_bc = const.tile([P, K], f32)
        nc.scalar.copy(out=mids_bc[:], in_=mids_ps[:])

        # ---- scores[p, c] = key[b, j*64+c, :] . mids[b, :] ----
        # 64 per-column 512 KB DMAs alternate rings; each column is consumed
        # by ONE fused multiply-reduce (native scalar_tensor_tensor with
        # accum_out).  DVE handles 3 of 4 columns, GpSimd the 4th, so the
        # combined rate beats the ~1.22 us/col arrival rate with slack.
        scores_sb = const.tile([P, CF], f32)
        tanh_sb = const.tile([P, CF], f32)
        em_sb = const.tile([P, CF], f32)
        key_r = key_e[:].rearrange("b (j c) k -> (b j) c k", j=J)
        GRP_COLS = 16  # tanh/exp epilogue group size
        for c in range(CF):
            eng = nc.sync if c % 2 == 0 else nc.scalar
            kt = kpool.tile([P, K], f32, tag="k")
            eng.dma_start(out=kt[:], in_=key_r[:, c, :])
            prod = vpool.tile([P, K], f32, tag="v")
            nc.vector.scalar_tensor_tensor(
                out=prod[:],
                in0=kt[:],
                scalar=0.0,
                in1=mids_bc[:],
                op0=byp,
                op1=mul,
                accum_out=scores_sb[:, c : c + 1],
            )
            # incremental tanh+exp per completed 16-col group (ACT engine,
            # overlapped with the stream; only the last group is tail work)
            if c % GRP_COLS == GRP_COLS - 1:
                g0 = c + 1 - GRP_COLS
                nc.scalar.activation(
                    out=tanh_sb[:, g0 : c + 1],
                    in_=scores_sb[:, g0 : c + 1],
                    func=mybir.ActivationFunctionType.Tanh,
                    bias=bias_sb[:],
                    scale=1.0,
                )
                nc.scalar.activation(
                    out=em_sb[:, g0 : c + 1],
                    in_=tanh_sb[:, g0 : c + 1],
                    func=mybir.ActivationFunctionType.Exp,
                )

        # ---- epilogue: mask, normalize ----
        emm_sb = const.tile([P, CF], f32)
        rowsum = const.tile([P, 1], f32)
        nc.vector.scalar_tensor_tensor(
            out=emm_sb[:],
            in0=em_sb[:],
            scalar=0.0,
            in1=maskr_sb[:],
            op0=byp,
            op1=mul,
            accum_out=rowsum[:],
        )
        den_ps = psum.tile([P, 1], f32)
        nc.tensor.matmul(
            den_ps[:], lhsT=grp_sb[:], rhs=rowsum[:], start=True, stop=True
        )
        rinv = const.tile([P, 1], f32)
        nc.vector.reciprocal(out=rinv[:], in_=den_ps[:])
        attn_sb = const.tile([P, CF], f32)
        nc.vector.tensor_scalar_mul(attn_sb[:], emm_sb[:], rinv[:])
        nc.sync.dma_start(out=out_e[:], in_=attn_sb[:])

    nc.compile()
    return nc


def _get_nc():
    if "nc" not in _STATE:
        _STATE["nc"] = _build_nc()
    return _STATE["nc"]


def _grp():
    if "GRP" not in _STATE:
        # GRP[p, m] = 1 iff p // J == m // J  (block-diagonal group-sum)
        pj = np.arange(P) // J
        _STATE["GRP"] = np.ascontiguousarray(
            (pj[:, None] == pj[None, :]).astype(np.float32)
        )
    return _STATE["GRP"]


def _make_in_maps(query, key, mask, W, bias):
    query = np.asarray(query, dtype=np.float32)
    key = np.asarray(key, dtype=np.float32)
    mask = np.asarray(mask, dtype=np.float32)
    W = np.asarray(W, dtype=np.float32)
    bias = np.asarray(bias, dtype=np.float32).reshape(-1)

    # wt[p, qc, k] = W.T[qc*128 + p, k]
    WT = np.ascontiguousarray(
        np.ascontiguousarray(W.T).reshape(QC, P, K).transpose(1, 0, 2)
    )
    GRP = _grp()
    biasb = np.ascontiguousarray(
        np.broadcast_to(bias[:1][None, :], (P, 1)).astype(np.float32)
    )

    in_maps = []
    for i in range(NCORES):
        sh = slice(i * BS, (i + 1) * BS)
        in_maps.append(
            {
                # pre-laid [P, QC, BS]: qt[p, qc, b] = query[sh].T[qc*128+p, b]
                "qt": np.ascontiguousarray(
                    query[sh].T.reshape(QC, P, BS).transpose(1, 0, 2)
                ),
                "wt": WT,
                "grp": GRP,
                "key": np.ascontiguousarray(key[sh]),
                "maskr": np.ascontiguousarray(mask[sh]).reshape(P, CF),
                "biasb": biasb,
            }
        )
    return in_maps


def _run(in_maps, **kwargs):
    from concourse.bass_utils import run_bass_kernel_spmd

    return run_bass_kernel_spmd(
        _get_nc(), in_maps, core_ids=list(range(NCORES)), **kwargs
    )


def _gather(results):
    return np.concatenate(
        [np.asarray(r["out"]).reshape(BS, T) for r in results], axis=0
    )


def kernel(query, key, mask, W, bias):
    in_maps = _make_in_maps(query, key, mask, W, bias)
    res = _run(in_maps)
    return _gather(res.results)


# revision 9
# speedup vs baseline: 1.1130x; 1.1130x over previous
"""Trainium2 Bass kernel for masked-softmax attention scoring.

Reference computation (B=128, T=512, K=1024, Q=1024):
    mids  = einsum("kq,bq->bk", W, query)
    s     = tanh(einsum("btk,bk->bt", key, mids) + bias)
    attn  = softmax-like: exp(s - max) * mask / sum(exp(s - max) * mask)

The max-subtraction cancels exactly in the ratio (tanh is bounded), so the
device computes  attn = exp(tanh(.)) * mask / sum_t(exp(tanh(.)) * mask).

Sharding: data-parallel over B across 8 NeuronCores (16 batches/core).
Per-core layout: partition p = (b, j) with b in [0,16), j in [0,8);
free column c in [0,64); timestep t = j*64 + c.

v2 design (DMA-bound pipeline, ~420 GB/s/core fabric):
 - W^T prologue split across both HWDGE rings, mids matmul in chunk
   arrival order (stationary = 8x-replicated query columns, f32r).
 - key streams as 64 per-column 512 KB DMAs alternating rings into a
   20-deep column pool; consumption is the NATIVE fused
   scalar_tensor_tensor (out = key*mids, accum_out = per-col score):
   DVE takes 3 of every 4 columns (~1.2 us/col), GpSimd every 4th
   (~2.5 us/col) so combined consumption always outruns arrival and the
   DMA rings never stall on pool recycling.
 - tanh/exp run incrementally per 16-column group on the ACT engine;
   the final mask-multiply + row-sum is one DVE op, group-sum via
   block-diagonal 0/1 matmul, reciprocal, scale, store.
"""

import sys

if "/opt/trn_rl_repo" not in sys.path:
    sys.path.insert(0, "/opt/trn_rl_repo")

from contextlib import ExitStack

import numpy as np

# ---- problem constants (hardcoded per spec) ----
B, T, K, Q = 128, 512, 1024, 1024
NCORES = 8
BS = B // NCORES          # 16 batches per core
P = 128                   # SBUF partitions
J = P // BS               # 8 t-blocks per batch on partitions
CF = T // J               # 64 timesteps per (partition, free col)
QC = Q // P               # 8 contraction chunks for the mids matmul
KEY_BUFS = 6              # key chunk pool depth (2 MB per slot)

_STATE: dict = {}


def _build_nc():
    import concourse.tile as tile
    from concourse import bacc, mybir

    f32 = mybir.dt.float32
    f32r = mybir.dt.float32r
    mul = mybir.AluOpType.mult
    byp = mybir.AluOpType.bypass
    nc = bacc.Bacc()

    qt_e = nc.declare_dram_parameter("qt", [P, QC, BS], f32r, isOutput=False)
    wt_e = nc.declare_dram_parameter("wt", [P, QC, K], f32r, isOutput=False)
    grp_e = nc.declare_dram_parameter("grp", [P, P], f32, isOutput=False)
    key_e = nc.declare_dram_parameter("key", [BS, T, K], f32, isOutput=False)
    maskr_e = nc.declare_dram_parameter("maskr", [P, CF], f32, isOutput=False)
    bias_e = nc.declare_dram_parameter("biasb", [P, 1], f32, isOutput=False)
    out_e = nc.declare_dram_parameter("out", [P, CF], f32, isOutput=True)

    with tile.TileContext(nc) as tc, ExitStack() as ctx:
        const = ctx.enter_context(tc.tile_pool(name="const", bufs=1))
        kpool = ctx.enter_context(tc.tile_pool(name="key", bufs=KEY_BUFS))
        vpool = ctx.enter_context(tc.tile_pool(name="vprod", bufs=2))
        psum = ctx.enter_context(tc.tile_pool(name="psum", bufs=1, space="PSUM"))

        # ---- prologue loads split across BOTH HWDGE rings so W^T lands
        # at aggregate fabric bandwidth; key columns queue right behind.
        qt_sb = const.tile([P, QC, BS], f32r)
        nc.sync.dma_start(out=qt_sb[:], in_=qt_e[:])
        maskr_sb = const.tile([P, CF], f32)
        nc.scalar.dma_start(out=maskr_sb[:], in_=maskr_e[:])
        bias_sb = const.tile([P, 1], f32)
        nc.scalar.dma_start(out=bias_sb[:], in_=bias_e[:])
        grp_sb = const.tile([P, P], f32)
        nc.scalar.dma_start(out=grp_sb[:], in_=grp_e[:])
        wt_sb = const.tile([P, QC, K], f32r)
        for qc in range(QC // 2):
            nc.sync.dma_start(out=wt_sb[:, qc, :], in_=wt_e[:, qc, :])
        for qc in range(QC // 2, QC):
            nc.scalar.dma_start(out=wt_sb[:, qc, :], in_=wt_e[:, qc, :])

        # ---- mids in broadcast layout: [P, K], row p = mids[b(p), :] ----
        # Replicate each query column 8x on-chip (stride-0 DVE read) so the
        # stationary operand has the (b, j) partition order in one free dim.
        qtrep_sb = const.tile([P, QC, BS, J], f32r)
        nc.gpsimd.tensor_copy(
            qtrep_sb[:], qt_sb[:].unsqueeze(-1).broadcast_to((P, QC, BS, J))
        )
        # matmuls in wt-chunk ARRIVAL order (rings deliver 0..3 and 4..7
        # concurrently); PSUM accumulation order is numerically immaterial.
        mids_ps = psum.tile([P, K], f32)
        qc_order = [0, 4, 1, 5, 2, 6, 3, 7]
        for qi, qc in enumerate(qc_order):
            lhsT = qtrep_sb[:, qc, :, :]
            for h in range(2):
                nc.tensor.matmul(
                    mids_ps[:, h * 512 : (h + 1) * 512],
                    lhsT=lhsT,
                    rhs=wt_sb[:, qc, h * 512 : (h + 1) * 512],
                    start=(qi == 0),
                    stop=(qi == QC - 1),
                )
        mids_bc = const.tile([P, K], f32)
        nc.scalar.copy(out=mids_bc[:], in_=mids_ps[:])

        # ---- scores[p, c] = key[b, j*64+c, :] . mids[b, :] ----
        # 64 per-column 512 KB DMAs alternate rings; each column is consumed
        # by ONE fused multiply-reduce (native scalar_tensor_tensor with
        # accum_out).  DVE handles 3 of 4 columns, GpSimd the 4th, so the
        # combined rate beats the ~1.22 us/col arrival rate with slack.
        scores_sb = const.tile([P, CF], f32)
        tanh_sb = const.tile([P, CF], f32)
        em_sb = const.tile([P, CF], f32)
        key_r = key_e[:].rearrange("b (j c) k -> (b j) c k", j=J)
        GRP_COLS = 16  # tanh/exp epilogue group size
        # chunk layout: 4-col 2 MB DMAs sustain full fabric rate; the last
        # 4 columns go as single-col DMAs so the tail semaphore granularity
        # is one column.
        chunks = [(c0, 4) for c0 in range(0, CF - 4, 4)] + [
            (c0, 1) for c0 in range(CF - 4, CF)
        ]
        for ci, (c0, sz) in enumerate(chunks):
            eng = nc.sync if ci % 2 == 0 else nc.scalar
            kt = kpool.tile([P, 4, K], f32, tag="k")
            eng.dma_start(out=kt[:, 0:sz, :], in_=key_r[:, c0 : c0 + sz, :])
            for cc in range(sz):
                c = c0 + cc
                prod = vpool.tile([P, K], f32, tag="v")
                nc.vector.scalar_tensor_tensor(
                    out=prod[:],
                    in0=kt[:, cc, :],
                    scalar=0.0,
                    in1=mids_bc[:],
                    op0=byp,
                    op1=mul,
                    accum_out=scores_sb[:, c : c + 1],
                )
            # incremental tanh+exp per completed 16-col group (ACT engine,
            # overlapped with the stream; only the last group is tail work)
            if c % GRP_COLS == GRP_COLS - 1:
                g0 = c + 1 - GRP_COLS
                nc.scalar.activation(
                    out=tanh_sb[:, g0 : c + 1],
                    in_=scores_sb[:, g0 : c + 1],
                    func=mybir.ActivationFunctionType.Tanh,
                    bias=bias_sb[:],
                    scale=1.0,
                )
                nc.scalar.activation(
                    out=em_sb[:, g0 : c + 1],
                    in_=tanh_sb[:, g0 : c + 1],
                    func=mybir.ActivationFunctionType.Exp,
                )

        # ---- epilogue: mask, normalize ----
        emm_sb = const.tile([P, CF], f32)
        rowsum = const.tile([P, 1], f32)
        nc.vector.scalar_tensor_tensor(
            out=emm_sb[:],
            in0=em_sb[:],
            scalar=0.0,
            in1=maskr_sb[:],
            op0=byp,
            op1=mul,
            accum_out=rowsum[:],
        )
        den_ps = psum.tile([P, 1], f32)
        nc.tensor.matmul(
            den_ps[:], lhsT=grp_sb[:], rhs=rowsum[:], start=True, stop=True
        )
        rinv = const.tile([P, 1], f32)
        nc.vector.reciprocal(out=rinv[:], in_=den_ps[:])
        attn_sb = const.tile([P, CF], f32)
        nc.vector.tensor_scalar_mul(attn_sb[:], emm_sb[:], rinv[:])
        nc.sync.dma_start(out=out_e[:], in_=attn_sb[:])

    nc.compile()
    return nc


def _get_nc():
    if "nc" not in _STATE:
        _STATE["nc"] = _build_nc()
    return _STATE["nc"]


def _grp():
    if "GRP" not in _STATE:
        # GRP[p, m] = 1 iff p // J == m // J  (block-diagonal group-sum)
        pj = np.arange(P) // J
        _STATE["GRP"] = np.ascontiguousarray(
            (pj[:, None] == pj[None, :]).astype(np.float32)
        )
    return _STATE["GRP"]


def _make_in_maps(query, key, mask, W, bias):
    query = np.asarray(query, dtype=np.float32)
    key = np.asarray(key, dtype=np.float32)
    mask = np.asarray(mask, dtype=np.float32)
    W = np.asarray(W, dtype=np.float32)
    bias = np.asarray(bias, dtype=np.float32).reshape(-1)

    # wt[p, qc, k] = W.T[qc*128 + p, k]
    WT = np.ascontiguousarray(
        np.ascontiguousarray(W.T).reshape(QC, P, K).transpose(1, 0, 2)
    )
    GRP = _grp()
    biasb = np.ascontiguousarray(
        np.broadcast_to(bias[:1][None, :], (P, 1)).astype(np.float32)
    )

    in_maps = []
    for i in range(NCORES):
        sh = slice(i * BS, (i + 1) * BS)
        in_maps.append(
            {
                # pre-laid [P, QC, BS]: qt[p, qc, b] = query[sh].T[qc*128+p, b]
                "qt": np.ascontiguousarray(
                    query[sh].T.reshape(QC, P, BS).transpose(1, 0, 2)
                ),
                "wt": WT,
                "grp": GRP,
                "key": np.ascontiguousarray(key[sh]),
                "maskr": np.ascontiguousarray(mask[sh]).reshape(P, CF),
                "biasb": biasb,
            }
        )
    return in_maps


def _run(in_maps, **kwargs):
    from concourse.bass_utils import run_bass_kernel_spmd

    return run_bass_kernel_spmd(
        _get_nc(), in_maps, core_ids=list(range(NCORES)), **kwargs
    )


def _gather(results):
    return np.concatenate(
        [np.asarray(r["out"]).reshape(BS, T) for r in results], axis=0
    )


def kernel(query, key, mask, W, bias):
    in_maps = _make_in_maps(query, key, mask, W, bias)
    res = _run(in_maps)
    return _gather(res.results)
